# revision 1
# baseline (speedup 1.0000x reference)
"""Causal multi-head self-attention with RoPE on 8 NeuronCores.

Sharding (hardcoded): core c -> batch b = c // 2, head-group hg = c % 2.
Each core:
  - projects its batch's x with column-sharded WQ/WK/WV (8 heads = 512 dims),
  - applies RoPE (host-precomputed cos/sin tables, adjacent-pair swap done
    on-chip with stream_shuffle),
  - runs causal attention for its 8 heads in transposed layout
    (S^T = [k, q]; softmax denominator comes free from a ones-column
    appended to V; normalization is broadcast via a K=1 matmul),
  - applies the row-sharded WO projection -> partial [T, D] output.
Host sums the two partials per batch (the "all-reduce after WO").
"""

import numpy as np
import ml_dtypes

B, T, D, H = 4, 2048, 1024, 16
DK = 64
HLOC = 8          # heads per core
E = HLOC * DK     # 512, local projection width
NCORES = 8
THETA = 10000.0

_BF16 = ml_dtypes.bfloat16

_cache = {}


def _build(t=T, hloc=HLOC, d=D, reps=1, psum_layout=0):
    from contextlib import ExitStack

    import concourse.bacc as bacc
    import concourse.bass as bass  # noqa: F401
    import concourse.mybir as mybir
    import concourse.tile as tile

    f32 = mybir.dt.float32
    bf16 = mybir.dt.bfloat16
    Exp = mybir.ActivationFunctionType.Exp

    e = hloc * DK
    npair = hloc // 2       # head-pair tiles in QT/KT/OT
    dsub = d // 128         # contraction subtiles for projections
    tq = t // 512           # 512-wide q chunks
    tk = t // 128           # 128-wide k tiles
    ep = e // 128           # output-partition tiles for Q/K (= npair)
    swap_mask = [i ^ 1 for i in range(32)]
    e_v = min(512, e)

    nc = bacc.Bacc("TRN2", target_bir_lowering=False, debug=False)

    xT = nc.declare_dram_parameter("xT", [d, t], bf16, False).ap()
    wqT = nc.declare_dram_parameter("wqT", [d, e], bf16, False).ap()
    wkT = nc.declare_dram_parameter("wkT", [d, e], bf16, False).ap()
    wvT = nc.declare_dram_parameter("wvT", [d, e], bf16, False).ap()
    woT = nc.declare_dram_parameter("woT", [e, d], bf16, False).ap()
    cosT = nc.declare_dram_parameter("cosT", [128, t], f32, False).ap()
    sinT = nc.declare_dram_parameter("sinT", [128, t], f32, False).ap()
    trim = nc.declare_dram_parameter("trim", [128, 128], bf16, False).ap()
    y = nc.declare_dram_parameter("y", [t, d], f32, True).ap()
    scr = nc.dram_tensor("scr", [hloc, t], bf16)  # denom-recip bounce for bcast

    with tile.TileContext(nc) as tc:
        with ExitStack() as ctx:
            const = ctx.enter_context(tc.tile_pool(name="const", bufs=1))
            ptpool = ctx.enter_context(tc.tile_pool(name="ptp", bufs=8))
            normp = ctx.enter_context(tc.tile_pool(name="normp", bufs=4))
            ysbp = ctx.enter_context(tc.tile_pool(name="ysbp", bufs=3))

            wq_sb = const.tile([128, dsub, e], bf16)
            wk_sb = const.tile([128, dsub, e], bf16)
            wv_sb = const.tile([128, dsub, e], bf16)
            wo_sb = const.tile([128, e // 128, d], bf16)
            trim_sb = const.tile([128, 128], bf16)
            qt_sb = const.tile([128, npair, t], bf16)
            kt_sb = const.tile([128, npair, t], bf16)
            v_sb = const.tile([128, tk, hloc, DK + 1], bf16)
            ot_sb = const.tile([128, npair, t], bf16)

            # split the big constant loads per d-subtile so they spread
            # across DMA queues (a single dma_start rides one queue) and so
            # the first projection matmuls can start after ~0.5 MB
            xt_sb = const.tile([128, dsub, t], bf16)
            cos_sb = const.tile([128, t], f32)
            sin_sb = const.tile([128, t], f32)
            xT4 = xT.rearrange("(n p) t -> p n t", p=128)
            wq4 = wqT.rearrange("(n p) e -> p n e", p=128)
            wk4 = wkT.rearrange("(n p) e -> p n e", p=128)
            wv4 = wvT.rearrange("(n p) e -> p n e", p=128)
            for ds_ in range(dsub):
                nc.sync.dma_start(wq_sb[:, ds_, :], wq4[:, ds_, :])
                nc.sync.dma_start(xt_sb[:, ds_, :], xT4[:, ds_, :])
                nc.sync.dma_start(wk_sb[:, ds_, :], wk4[:, ds_, :])
                nc.sync.dma_start(wv_sb[:, ds_, :], wv4[:, ds_, :])
            wo4 = woT.rearrange("(n p) d -> p n d", p=128)
            for dp in range(e // 128):
                nc.sync.dma_start(wo_sb[:, dp, :], wo4[:, dp, :])
            nc.sync.dma_start(trim_sb, trim)
            nc.vector.memset(v_sb[:, :, :, DK : DK + 1], 1.0)
            half = t // 2
            for hf in range(2):
                nc.sync.dma_start(
                    cos_sb[:, hf * half : (hf + 1) * half],
                    cosT[:, hf * half : (hf + 1) * half],
                )
                nc.sync.dma_start(
                    sin_sb[:, hf * half : (hf + 1) * half],
                    sinT[:, hf * half : (hf + 1) * half],
                )

            for _rep in range(reps):
                with ExitStack() as c1:
                    rope = c1.enter_context(tc.tile_pool(name="rope", bufs=3))
                    if psum_layout == 0:
                        ppsum = c1.enter_context(
                            tc.tile_pool(name="ppsum", bufs=1, space="PSUM")
                        )
                        spsum = c1.enter_context(
                            tc.tile_pool(name="spsum", bufs=2, space="PSUM")
                        )
                        opsum = c1.enter_context(
                            tc.tile_pool(name="opsum", bufs=2, space="PSUM")
                        )
                    elif psum_layout == 1:
                        spsum = c1.enter_context(
                            tc.tile_pool(name="spsum", bufs=2, space="PSUM")
                        )
                        opsum = c1.enter_context(
                            tc.tile_pool(name="opsum", bufs=4, space="PSUM")
                        )
                        ppsum = spsum
                    else:
                        spsum = c1.enter_context(
                            tc.tile_pool(name="spsum", bufs=3, space="PSUM")
                        )
                        opsum = c1.enter_context(
                            tc.tile_pool(name="opsum", bufs=2, space="PSUM")
                        )
                        ppsum = spsum

                    # ---- V projection (psum via the shared "s" slots) ----
                    for it in range(tk):
                        v_ps = spsum.tile([128, 1024], f32, tag="s")
                        for ds_ in range(dsub):
                            nc.tensor.matmul(
                                v_ps[:, :e_v],
                                lhsT=xt_sb[:, ds_, it * 128 : (it + 1) * 128],
                                rhs=wv_sb[:, ds_, :e_v],
                                start=(ds_ == 0),
                                stop=(ds_ == dsub - 1),
                            )
                        nc.vector.tensor_copy(
                            v_sb[:, it, :, 0:DK],
                            v_ps[:, :e_v].rearrange("p (h k) -> p h k", h=hloc),
                        )

                    ph = min(1024, t)          # projection t-chunk
                    nh = t // ph               # chunks per e-tile
                    for pr in range(npair):
                        # ---- Q/K projection + RoPE for this head pair ----
                        for wsb, dst in ((wq_sb, qt_sb), (wk_sb, kt_sb)):
                            for ch in range(nh):
                                q_ps = ppsum.tile(
                                    [128, ph],
                                    f32,
                                    tag="proj" if psum_layout == 0 else "s",
                                )
                                c0 = ch * ph
                                for ds_ in range(dsub):
                                    for jt in range(ph // 512):
                                        lo = c0 + jt * 512
                                        nc.tensor.matmul(
                                            q_ps[:, jt * 512 : (jt + 1) * 512],
                                            lhsT=wsb[:, ds_, pr * 128 : (pr + 1) * 128],
                                            rhs=xt_sb[:, ds_, lo : lo + 512],
                                            start=(ds_ == 0),
                                            stop=(ds_ == dsub - 1),
                                        )
                                # evict raw proj to sbuf, then RoPE from sbuf
                                raw = rope.tile([128, ph], f32, tag="raw")
                                nc.vector.tensor_copy(raw, q_ps[:, :])
                                sw = rope.tile([128, ph], f32, tag="sw")
                                nc.vector.stream_shuffle(sw, raw, mask=swap_mask)
                                nc.vector.tensor_mul(
                                    dst[:, pr, c0 : c0 + ph],
                                    raw,
                                    cos_sb[:, c0 : c0 + ph],
                                )
                                nc.gpsimd.tensor_mul(sw, sw, sin_sb[:, c0 : c0 + ph])
                                nc.gpsimd.tensor_add(
                                    dst[:, pr, c0 : c0 + ph],
                                    dst[:, pr, c0 : c0 + ph],
                                    sw,
                                )

                        # ---- attention for heads (2pr, 2pr+1), q-subtiles ----
                        hA, hB = 2 * pr, 2 * pr + 1
                        for j in range(tq):
                            o_a = opsum.tile([128, 512], f32, tag="o")
                            o_b = opsum.tile([128, 512], f32, tag="o")
                            for i in range(4 * j + 4):
                                off = 128 * (i % 4) if i // 4 == j else 0
                                qlo = j * 512
                                s_ps = spsum.tile([128, 1024], f32, tag="s")
                                # diagonal tiles: stream only q-columns >= off
                                # (below-diagonal columns are all-zero in P
                                # and never touched anywhere downstream)
                                nc.tensor.matmul(
                                    s_ps[:, off:512],
                                    lhsT=kt_sb[0:64, pr, i * 128 : (i + 1) * 128],
                                    rhs=qt_sb[0:64, pr, qlo + off : (j + 1) * 512],
                                    start=True,
                                    stop=True,
                                )
                                nc.tensor.matmul(
                                    s_ps[:, 512 + off : 1024],
                                    lhsT=kt_sb[64:128, pr, i * 128 : (i + 1) * 128],
                                    rhs=qt_sb[64:128, pr, qlo + off : (j + 1) * 512],
                                    start=True,
                                    stop=True,
                                )
                                pt = ptpool.tile([128, 1024], bf16, tag="pt")
                                if off == 0:
                                    nc.scalar.activation(
                                        pt[:, :], s_ps[:, :], Exp, scale=DK ** (-0.5)
                                    )
                                else:
                                    nc.scalar.activation(
                                        pt[:, off:512],
                                        s_ps[:, off:512],
                                        Exp,
                                        scale=DK ** (-0.5),
                                    )
                                    nc.scalar.activation(
                                        pt[:, 512 + off : 1024],
                                        s_ps[:, 512 + off : 1024],
                                        Exp,
                                        scale=DK ** (-0.5),
                                    )
                                if i // 4 == j:
                                    nc.gpsimd.tensor_mul(
                                        pt[:, off : off + 128],
                                        pt[:, off : off + 128],
                                        trim_sb,
                                    )
                                    nc.gpsimd.tensor_mul(
                                        pt[:, 512 + off : 512 + off + 128],
                                        pt[:, 512 + off : 512 + off + 128],
                                        trim_sb,
                                    )
                                nc.tensor.matmul(
                                    o_a[0:65, off:512],
                                    lhsT=v_sb[:, i, hA, :],
                                    rhs=pt[:, off:512],
                                    start=(i == 0),
                                    stop=(i == 4 * j + 3),
                                )
                                nc.tensor.matmul(
                                    o_b[0:65, off:512],
                                    lhsT=v_sb[:, i, hB, :],
                                    rhs=pt[:, 512 + off : 1024],
                                    start=(i == 0),
                                    stop=(i == 4 * j + 3),
                                )
                            # normalize both heads for this q-subtile
                            for hx, o_ps, po in ((hA, o_a, 0), (hB, o_b, 64)):
                                recip = normp.tile([1, 512], bf16, tag="recip")
                                with nc.allow_low_precision(
                                    reason="softmax recip bf16"
                                ):
                                    nc.vector.reciprocal(
                                        recip[0:1, :], o_ps[64:65, 0:512]
                                    )
                                nc.sync.dma_start(
                                    scr[hx : hx + 1, j * 512 : (j + 1) * 512],
                                    recip[0:1, :],
                                )
                                bc_sb = normp.tile([64, 512], bf16, tag="bc")
                                scr_bcast = bass.AP(
                                    tensor=scr.ap().tensor,
                                    offset=hx * t + j * 512,
                                    ap=[[0, 64], [1, 512]],
                                )
                                nc.sync.dma_start(bc_sb, scr_bcast)
                                nc.vector.tensor_mul(
                                    ot_sb[po : po + 64, pr, j * 512 : (j + 1) * 512],
                                    o_ps[0:64, 0:512],
                                    bc_sb,
                                )

                    # ---- WO projection (psum via the shared "s" slots) ----
                    nech = d // 512
                    for it in range(tk):
                        y_sb = ysbp.tile([128, d], f32, tag="ysb")
                        for ec in range(nech):
                            y_ps = spsum.tile([128, 1024], f32, tag="s")
                            for dp in range(e // 128):
                                nc.tensor.matmul(
                                    y_ps[:, 0:512],
                                    lhsT=ot_sb[:, dp, it * 128 : (it + 1) * 128],
                                    rhs=wo_sb[:, dp, ec * 512 : (ec + 1) * 512],
                                    start=(dp == 0),
                                    stop=(dp == e // 128 - 1),
                                )
                            nc.vector.tensor_copy(
                                y_sb[:, ec * 512 : (ec + 1) * 512], y_ps[:, 0:512]
                            )
                        nc.sync.dma_start(y[it * 128 : (it + 1) * 128, :], y_sb)

    nc.compile()
    return nc


def _get_nc():
    if "nc" not in _cache:
        _cache["nc"] = _build()
    return _cache["nc"]


def _host_tables(positions):
    """cos/sin RoPE tables laid out for the on-chip [128, T] tiles."""
    pos = np.asarray(positions, np.float32)  # [t]
    inv = 1.0 / THETA ** (
        (2.0 * np.arange(1, DK // 2 + 1, dtype=np.float32) - 2.0) / DK
    )  # [32]
    ang = pos[None, :] * inv[:, None]  # [32, t]
    c32 = np.cos(ang)
    s32 = np.sin(ang)
    rows = np.arange(128)
    dloc = rows % DK
    fidx = dloc // 2
    sign = np.where(dloc % 2 == 0, -1.0, 1.0).astype(np.float32)
    cosT = c32[fidx, :]
    sinT = sign[:, None] * s32[fidx, :]
    return np.ascontiguousarray(cosT), np.ascontiguousarray(sinT)


def _make_in_maps(inputs):
    x = np.asarray(inputs["x"], np.float32)
    token_positions = np.asarray(inputs["token_positions"])
    WQ = np.asarray(inputs["WQ"], np.float32)
    WK = np.asarray(inputs["WK"], np.float32)
    WV = np.asarray(inputs["WV"], np.float32)
    WO = np.asarray(inputs["WO"], np.float32)
    trimask = np.triu(np.ones((128, 128), np.float32)).astype(_BF16)

    # per-head-group weight shards (shared across batches)
    wsh = {}
    for hg in range(2):
        sl = slice(hg * E, (hg + 1) * E)
        wsh[hg] = {
            "wqT": np.ascontiguousarray(WQ[sl, :].T).astype(_BF16),
            "wkT": np.ascontiguousarray(WK[sl, :].T).astype(_BF16),
            "wvT": np.ascontiguousarray(WV[sl, :].T).astype(_BF16),
            "woT": np.ascontiguousarray(WO[:, sl].T).astype(_BF16),
        }
    xts = {b: np.ascontiguousarray(x[b].T).astype(_BF16) for b in range(B)}
    tabs = {}
    for b in range(B):
        key = token_positions[b].tobytes()
        if key not in tabs:
            tabs[key] = _host_tables(token_positions[b])

    in_maps = []
    for c in range(NCORES):
        b, hg = c // 2, c % 2
        cosT, sinT = tabs[token_positions[b].tobytes()]
        in_maps.append(
            {
                "xT": xts[b],
                "cosT": cosT,
                "sinT": sinT,
                "trim": trimask,
                **wsh[hg],
            }
        )
    return in_maps


def _get_runner():
    """Build (once) a jitted shard_map over the 8 cores for the bass program."""
    if "runner" in _cache:
        return _cache["runner"]

    import jax
    from jax.sharding import Mesh, PartitionSpec
    from jax.experimental.shard_map import shard_map
    from concourse import bass2jax
    from concourse.bass2jax import _bass_exec_p, partition_id_tensor
    import concourse.mybir as mybir

    bass2jax.install_neuronx_cc_hook()
    nc = _get_nc()

    partition_name = nc.partition_id_tensor.name if nc.partition_id_tensor else None
    in_names, out_names, out_avals, zero_outs = [], [], [], []
    for alloc in nc.m.functions[0].allocations:
        if not isinstance(alloc, mybir.MemoryLocationSet):
            continue
        name = alloc.memorylocations[0].name
        if alloc.kind == "ExternalInput":
            if name != partition_name:
                in_names.append(name)
        elif alloc.kind == "ExternalOutput":
            out_names.append(name)
            np_dt = mybir.dt.np(alloc.dtype)
            out_avals.append(jax.core.ShapedArray(tuple(alloc.tensor_shape), np_dt))
            zero_outs.append(np.zeros(tuple(alloc.tensor_shape), np_dt))
    all_names = in_names + out_names
    if partition_name is not None:
        all_names = all_names + [partition_name]

    def _body(*args):
        operands = list(args)
        if partition_name is not None:
            operands.append(partition_id_tensor())
        return tuple(
            _bass_exec_p.bind(
                *operands,
                out_avals=tuple(out_avals),
                in_names=tuple(all_names),
                out_names=tuple(out_names),
                lowering_input_output_aliases=(),
                sim_require_finite=True,
                sim_require_nnan=True,
                nc=nc,
            )
        )

    devices = jax.devices()[:NCORES]
    mesh = Mesh(np.asarray(devices), ("core",))
    nin = len(in_names) + len(out_names)
    sharded = jax.jit(
        shard_map(
            _body,
            mesh=mesh,
            in_specs=(PartitionSpec("core"),) * nin,
            out_specs=(PartitionSpec("core"),) * len(out_names),
            check_rep=False,
        ),
        keep_unused=True,
    )
    concat_zeros = [
        np.zeros((NCORES * z.shape[0], *z.shape[1:]), z.dtype) for z in zero_outs
    ]
    _cache["runner"] = (sharded, in_names, out_names, concat_zeros)
    return _cache["runner"]


def kernel(x, token_positions, WQ, WK, WV, WO):
    in_maps = _make_in_maps(
        {
            "x": x,
            "token_positions": token_positions,
            "WQ": WQ,
            "WK": WK,
            "WV": WV,
            "WO": WO,
        }
    )
    sharded, in_names, out_names, concat_zeros = _get_runner()
    concat_in = [
        np.concatenate([np.asarray(in_maps[c][nm]) for c in range(NCORES)], axis=0)
        for nm in in_names
    ]
    out_arrs = sharded(*concat_in, *concat_zeros)
    ys = np.asarray(out_arrs[out_names.index("y")]).reshape(NCORES, T, D)
    out = np.empty((B, T, D), np.float32)
    for b in range(B):
        out[b] = ys[2 * b] + ys[2 * b + 1]
    return out



# revision 4
# speedup vs baseline: 2.3755x; 2.3755x over previous
"""Causal multi-head self-attention with RoPE on 8 NeuronCores.

Sharding (hardcoded): core c -> batch b = c // 2, head-group hg = c % 2.
Each core:
  - projects its batch's x with column-sharded WQ/WK/WV (8 heads = 512 dims),
  - applies RoPE (host-precomputed cos/sin tables shipped as bf16 hi/lo
    pairs, summed on-chip to f32; adjacent-pair swap via stream_shuffle),
  - runs causal attention for its 8 heads in transposed layout
    (S^T = [k, q]; softmax denominator comes free from a ones-column
    appended to V; normalization is broadcast via a DRAM bounce),
  - applies the row-sharded WO projection -> partial [T, D] output.
Host sums the two partials per batch (the "all-reduce after WO").

All inputs are packed into ONE DRAM tensor per core: the per-call
execution overhead through the axon PJRT path scales with the number of
I/O buffers (~65us each), so 8 separate inputs cost ~0.5ms of pure
launch overhead. Packed layout (bf16, [3200, 2048]):
  rows    0:1024  xT (x[b].T, [d, t])
  rows 1024:2048  wqT | wkT | wvT  (cols 0:512, 512:1024, 1024:1536)
  rows 2048:2560  woT (cols 0:1024)
  rows 2560:3072  cos_hi | cos_lo | sin_hi | sin_lo ([128, 2048] each)
  rows 3072:3200  trim mask (cols 0:128)
"""

import numpy as np
import ml_dtypes

B, T, D, H = 4, 2048, 1024, 16
DK = 64
HLOC = 8          # heads per core
E = HLOC * DK     # 512, local projection width
NCORES = 8
THETA = 10000.0

_BF16 = ml_dtypes.bfloat16

# packed-input row offsets
_ROW_X = 0
_ROW_W = 1024
_ROW_WO = 2048
_ROW_TBL = 2560
_ROW_TRIM = 3072
_PK_ROWS = 3200
_PK_COLS = 2048

_cache = {}


def _build(t=T, hloc=HLOC, d=D, reps=1, psum_layout=0):
    from contextlib import ExitStack

    import concourse.bacc as bacc
    import concourse.bass as bass  # noqa: F401
    import concourse.mybir as mybir
    import concourse.tile as tile

    f32 = mybir.dt.float32
    bf16 = mybir.dt.bfloat16
    Exp = mybir.ActivationFunctionType.Exp

    e = hloc * DK
    npair = hloc // 2       # head-pair tiles in QT/KT/OT
    dsub = d // 128         # contraction subtiles for projections
    tq = t // 512           # 512-wide q chunks
    tk = t // 128           # 128-wide k tiles
    swap_mask = [i ^ 1 for i in range(32)]
    e_v = min(512, e)

    nc = bacc.Bacc("TRN2", target_bir_lowering=False, debug=False)

    pk = nc.declare_dram_parameter("pk", [_PK_ROWS, _PK_COLS], bf16, False).ap()
    y = nc.declare_dram_parameter("y", [t, d], f32, True).ap()
    scr = nc.dram_tensor("scr", [hloc, t], bf16)  # denom-recip bounce for bcast

    with tile.TileContext(nc) as tc:
        with ExitStack() as ctx:
            const = ctx.enter_context(tc.tile_pool(name="const", bufs=1))
            ptpool = ctx.enter_context(tc.tile_pool(name="ptp", bufs=8))
            normp = ctx.enter_context(tc.tile_pool(name="normp", bufs=4))
            ysbp = ctx.enter_context(tc.tile_pool(name="ysbp", bufs=3))

            wq_sb = const.tile([128, dsub, e], bf16)
            wk_sb = const.tile([128, dsub, e], bf16)
            wv_sb = const.tile([128, dsub, e], bf16)
            wo_sb = const.tile([128, e // 128, d], bf16)
            trim_sb = const.tile([128, 128], bf16)
            qt_sb = const.tile([128, npair, t], bf16)
            kt_sb = const.tile([128, npair, t], bf16)
            v_sb = const.tile([128, tk, hloc, DK + 1], bf16)
            ot_sb = const.tile([128, npair, t], bf16)

            # split the big constant loads per d-subtile so they spread
            # across DMA queues (a single dma_start rides one queue) and so
            # the first projection matmuls can start after ~0.5 MB
            xt_sb = const.tile([128, dsub, t], bf16)
            cos_sb = const.tile([128, t], f32)
            sin_sb = const.tile([128, t], f32)
            for ds_ in range(dsub):
                r = _ROW_W + ds_ * 128
                nc.sync.dma_start(wq_sb[:, ds_, :], pk[r : r + 128, 0:e])
                nc.sync.dma_start(
                    xt_sb[:, ds_, :], pk[ds_ * 128 : (ds_ + 1) * 128, :]
                )
                nc.sync.dma_start(wk_sb[:, ds_, :], pk[r : r + 128, e : 2 * e])
                nc.sync.dma_start(wv_sb[:, ds_, :], pk[r : r + 128, 2 * e : 3 * e])
            for dp in range(e // 128):
                r = _ROW_WO + dp * 128
                nc.sync.dma_start(wo_sb[:, dp, :], pk[r : r + 128, 0:d])
            nc.sync.dma_start(trim_sb, pk[_ROW_TRIM : _ROW_TRIM + 128, 0:128])
            nc.vector.memset(v_sb[:, :, :, DK : DK + 1], 1.0)
            with tc.tile_pool(name="tblp", bufs=1) as tblp:
                for dst, blk in ((cos_sb, 0), (sin_sb, 2)):
                    hi_sb = tblp.tile([128, t], bf16, tag="hi")
                    lo_sb = tblp.tile([128, t], bf16, tag="lo")
                    r = _ROW_TBL + blk * 128
                    nc.sync.dma_start(hi_sb, pk[r : r + 128, :])
                    nc.sync.dma_start(lo_sb, pk[r + 128 : r + 256, :])
                    nc.vector.tensor_add(dst, hi_sb, lo_sb)

            for _rep in range(reps):
                with ExitStack() as c1:
                    rope = c1.enter_context(tc.tile_pool(name="rope", bufs=3))
                    if psum_layout == 0:
                        ppsum = c1.enter_context(
                            tc.tile_pool(name="ppsum", bufs=1, space="PSUM")
                        )
                        spsum = c1.enter_context(
                            tc.tile_pool(name="spsum", bufs=2, space="PSUM")
                        )
                        opsum = c1.enter_context(
                            tc.tile_pool(name="opsum", bufs=2, space="PSUM")
                        )
                    elif psum_layout == 1:
                        spsum = c1.enter_context(
                            tc.tile_pool(name="spsum", bufs=2, space="PSUM")
                        )
                        opsum = c1.enter_context(
                            tc.tile_pool(name="opsum", bufs=4, space="PSUM")
                        )
                        ppsum = spsum
                    else:
                        spsum = c1.enter_context(
                            tc.tile_pool(name="spsum", bufs=3, space="PSUM")
                        )
                        opsum = c1.enter_context(
                            tc.tile_pool(name="opsum", bufs=2, space="PSUM")
                        )
                        ppsum = spsum

                    # ---- V projection (psum via the shared "s" slots) ----
                    for it in range(tk):
                        v_ps = spsum.tile([128, 1024], f32, tag="s")
                        for ds_ in range(dsub):
                            nc.tensor.matmul(
                                v_ps[:, :e_v],
                                lhsT=xt_sb[:, ds_, it * 128 : (it + 1) * 128],
                                rhs=wv_sb[:, ds_, :e_v],
                                start=(ds_ == 0),
                                stop=(ds_ == dsub - 1),
                            )
                        nc.vector.tensor_copy(
                            v_sb[:, it, :, 0:DK],
                            v_ps[:, :e_v].rearrange("p (h k) -> p h k", h=hloc),
                        )

                    ph = min(1024, t)          # projection t-chunk
                    nh = t // ph               # chunks per e-tile
                    for pr in range(npair):
                        # ---- Q/K projection + RoPE for this head pair ----
                        for wsb, dst in ((wq_sb, qt_sb), (wk_sb, kt_sb)):
                            for ch in range(nh):
                                q_ps = ppsum.tile(
                                    [128, ph],
                                    f32,
                                    tag="proj" if psum_layout == 0 else "s",
                                )
                                c0 = ch * ph
                                for ds_ in range(dsub):
                                    for jt in range(ph // 512):
                                        lo = c0 + jt * 512
                                        nc.tensor.matmul(
                                            q_ps[:, jt * 512 : (jt + 1) * 512],
                                            lhsT=wsb[:, ds_, pr * 128 : (pr + 1) * 128],
                                            rhs=xt_sb[:, ds_, lo : lo + 512],
                                            start=(ds_ == 0),
                                            stop=(ds_ == dsub - 1),
                                        )
                                # evict raw proj to sbuf, then RoPE from sbuf
                                raw = rope.tile([128, ph], f32, tag="raw")
                                nc.vector.tensor_copy(raw, q_ps[:, :])
                                sw = rope.tile([128, ph], f32, tag="sw")
                                nc.vector.stream_shuffle(sw, raw, mask=swap_mask)
                                nc.vector.tensor_mul(
                                    dst[:, pr, c0 : c0 + ph],
                                    raw,
                                    cos_sb[:, c0 : c0 + ph],
                                )
                                nc.gpsimd.tensor_mul(sw, sw, sin_sb[:, c0 : c0 + ph])
                                nc.gpsimd.tensor_add(
                                    dst[:, pr, c0 : c0 + ph],
                                    dst[:, pr, c0 : c0 + ph],
                                    sw,
                                )

                        # ---- attention for heads (2pr, 2pr+1), q-subtiles ----
                        hA, hB = 2 * pr, 2 * pr + 1
                        for j in range(tq):
                            o_a = opsum.tile([128, 512], f32, tag="o")
                            o_b = opsum.tile([128, 512], f32, tag="o")
                            for i in range(4 * j + 4):
                                off = 128 * (i % 4) if i // 4 == j else 0
                                qlo = j * 512
                                s_ps = spsum.tile([128, 1024], f32, tag="s")
                                # diagonal tiles: stream only q-columns >= off
                                # (below-diagonal columns are all-zero in P
                                # and never touched anywhere downstream)
                                nc.tensor.matmul(
                                    s_ps[:, off:512],
                                    lhsT=kt_sb[0:64, pr, i * 128 : (i + 1) * 128],
                                    rhs=qt_sb[0:64, pr, qlo + off : (j + 1) * 512],
                                    start=True,
                                    stop=True,
                                )
                                nc.tensor.matmul(
                                    s_ps[:, 512 + off : 1024],
                                    lhsT=kt_sb[64:128, pr, i * 128 : (i + 1) * 128],
                                    rhs=qt_sb[64:128, pr, qlo + off : (j + 1) * 512],
                                    start=True,
                                    stop=True,
                                )
                                pt = ptpool.tile([128, 1024], bf16, tag="pt")
                                if off == 0:
                                    nc.scalar.activation(
                                        pt[:, :], s_ps[:, :], Exp, scale=DK ** (-0.5)
                                    )
                                else:
                                    nc.scalar.activation(
                                        pt[:, off:512],
                                        s_ps[:, off:512],
                                        Exp,
                                        scale=DK ** (-0.5),
                                    )
                                    nc.scalar.activation(
                                        pt[:, 512 + off : 1024],
                                        s_ps[:, 512 + off : 1024],
                                        Exp,
                                        scale=DK ** (-0.5),
                                    )
                                if i // 4 == j:
                                    nc.gpsimd.tensor_mul(
                                        pt[:, off : off + 128],
                                        pt[:, off : off + 128],
                                        trim_sb,
                                    )
                                    nc.gpsimd.tensor_mul(
                                        pt[:, 512 + off : 512 + off + 128],
                                        pt[:, 512 + off : 512 + off + 128],
                                        trim_sb,
                                    )
                                nc.tensor.matmul(
                                    o_a[0:65, off:512],
                                    lhsT=v_sb[:, i, hA, :],
                                    rhs=pt[:, off:512],
                                    start=(i == 0),
                                    stop=(i == 4 * j + 3),
                                )
                                nc.tensor.matmul(
                                    o_b[0:65, off:512],
                                    lhsT=v_sb[:, i, hB, :],
                                    rhs=pt[:, 512 + off : 1024],
                                    start=(i == 0),
                                    stop=(i == 4 * j + 3),
                                )
                            # normalize both heads for this q-subtile
                            for hx, o_ps, po in ((hA, o_a, 0), (hB, o_b, 64)):
                                recip = normp.tile([1, 512], bf16, tag="recip")
                                with nc.allow_low_precision(
                                    reason="softmax recip bf16"
                                ):
                                    nc.vector.reciprocal(
                                        recip[0:1, :], o_ps[64:65, 0:512]
                                    )
                                nc.sync.dma_start(
                                    scr[hx : hx + 1, j * 512 : (j + 1) * 512],
                                    recip[0:1, :],
                                )
                                bc_sb = normp.tile([64, 512], bf16, tag="bc")
                                scr_bcast = bass.AP(
                                    tensor=scr.ap().tensor,
                                    offset=hx * t + j * 512,
                                    ap=[[0, 64], [1, 512]],
                                )
                                nc.sync.dma_start(bc_sb, scr_bcast)
                                nc.vector.tensor_mul(
                                    ot_sb[po : po + 64, pr, j * 512 : (j + 1) * 512],
                                    o_ps[0:64, 0:512],
                                    bc_sb,
                                )

                    # ---- WO projection (psum via the shared "s" slots) ----
                    nech = d // 512
                    for it in range(tk):
                        y_sb = ysbp.tile([128, d], f32, tag="ysb")
                        for ec in range(nech):
                            y_ps = spsum.tile([128, 1024], f32, tag="s")
                            for dp in range(e // 128):
                                nc.tensor.matmul(
                                    y_ps[:, 0:512],
                                    lhsT=ot_sb[:, dp, it * 128 : (it + 1) * 128],
                                    rhs=wo_sb[:, dp, ec * 512 : (ec + 1) * 512],
                                    start=(dp == 0),
                                    stop=(dp == e // 128 - 1),
                                )
                            nc.vector.tensor_copy(
                                y_sb[:, ec * 512 : (ec + 1) * 512], y_ps[:, 0:512]
                            )
                        nc.sync.dma_start(y[it * 128 : (it + 1) * 128, :], y_sb)

    nc.compile()
    return nc


def _get_nc():
    if "nc" not in _cache:
        _cache["nc"] = _build()
    return _cache["nc"]


def _host_tables(positions):
    """cos/sin RoPE tables laid out for the on-chip [128, T] tiles."""
    pos = np.asarray(positions, np.float32)  # [t]
    inv = 1.0 / THETA ** (
        (2.0 * np.arange(1, DK // 2 + 1, dtype=np.float32) - 2.0) / DK
    )  # [32]
    ang = pos[None, :] * inv[:, None]  # [32, t]
    c32 = np.cos(ang)
    s32 = np.sin(ang)
    rows = np.arange(128)
    dloc = rows % DK
    fidx = dloc // 2
    sign = np.where(dloc % 2 == 0, -1.0, 1.0).astype(np.float32)
    cosT = c32[fidx, :]
    sinT = sign[:, None] * s32[fidx, :]
    return np.ascontiguousarray(cosT), np.ascontiguousarray(sinT)


def _hi_lo(a):
    hi = a.astype(_BF16)
    lo = (a - hi.astype(np.float32)).astype(_BF16)
    return hi, lo


def _make_in_maps(inputs):
    x = np.asarray(inputs["x"], np.float32)
    token_positions = np.asarray(inputs["token_positions"])
    WQ = np.asarray(inputs["WQ"], np.float32)
    WK = np.asarray(inputs["WK"], np.float32)
    WV = np.asarray(inputs["WV"], np.float32)
    WO = np.asarray(inputs["WO"], np.float32)
    trimask = np.triu(np.ones((128, 128), np.float32)).astype(_BF16)

    # per-head-group weight shards (shared across batches)
    wsh = {}
    for hg in range(2):
        sl = slice(hg * E, (hg + 1) * E)
        wsh[hg] = (
            np.ascontiguousarray(WQ[sl, :].T).astype(_BF16),
            np.ascontiguousarray(WK[sl, :].T).astype(_BF16),
            np.ascontiguousarray(WV[sl, :].T).astype(_BF16),
            np.ascontiguousarray(WO[:, sl].T).astype(_BF16),
        )
    xts = {b: np.ascontiguousarray(x[b].T).astype(_BF16) for b in range(B)}
    tabs = {}
    for b in range(B):
        key = token_positions[b].tobytes()
        if key not in tabs:
            cosT, sinT = _host_tables(token_positions[b])
            tabs[key] = (*_hi_lo(cosT), *_hi_lo(sinT))

    in_maps = []
    for c in range(NCORES):
        b, hg = c // 2, c % 2
        chi, clo, shi, slo = tabs[token_positions[b].tobytes()]
        wq, wk, wv, wo = wsh[hg]
        pkb = np.zeros((_PK_ROWS, _PK_COLS), _BF16)
        pkb[_ROW_X : _ROW_X + D, :] = xts[b]
        pkb[_ROW_W : _ROW_W + D, 0:E] = wq
        pkb[_ROW_W : _ROW_W + D, E : 2 * E] = wk
        pkb[_ROW_W : _ROW_W + D, 2 * E : 3 * E] = wv
        pkb[_ROW_WO : _ROW_WO + E, 0:D] = wo
        pkb[_ROW_TBL : _ROW_TBL + 128, :] = chi
        pkb[_ROW_TBL + 128 : _ROW_TBL + 256, :] = clo
        pkb[_ROW_TBL + 256 : _ROW_TBL + 384, :] = shi
        pkb[_ROW_TBL + 384 : _ROW_TBL + 512, :] = slo
        pkb[_ROW_TRIM : _ROW_TRIM + 128, 0:128] = trimask
        in_maps.append({"pk": pkb})
    return in_maps


def _get_runner():
    """Build (once) a jitted shard_map over the 8 cores for the bass program."""
    if "runner" in _cache:
        return _cache["runner"]

    import jax
    from jax.sharding import Mesh, PartitionSpec
    from jax.experimental.shard_map import shard_map
    from concourse import bass2jax
    from concourse.bass2jax import _bass_exec_p, partition_id_tensor
    import concourse.mybir as mybir

    bass2jax.install_neuronx_cc_hook()
    nc = _get_nc()

    partition_name = nc.partition_id_tensor.name if nc.partition_id_tensor else None
    in_names, out_names, out_avals, zero_outs = [], [], [], []
    for alloc in nc.m.functions[0].allocations:
        if not isinstance(alloc, mybir.MemoryLocationSet):
            continue
        name = alloc.memorylocations[0].name
        if alloc.kind == "ExternalInput":
            if name != partition_name:
                in_names.append(name)
        elif alloc.kind == "ExternalOutput":
            out_names.append(name)
            np_dt = mybir.dt.np(alloc.dtype)
            out_avals.append(jax.core.ShapedArray(tuple(alloc.tensor_shape), np_dt))
            zero_outs.append(np.zeros(tuple(alloc.tensor_shape), np_dt))
    all_names = in_names + out_names
    if partition_name is not None:
        all_names = all_names + [partition_name]

    def _body(*args):
        operands = list(args)
        if partition_name is not None:
            operands.append(partition_id_tensor())
        return tuple(
            _bass_exec_p.bind(
                *operands,
                out_avals=tuple(out_avals),
                in_names=tuple(all_names),
                out_names=tuple(out_names),
                lowering_input_output_aliases=(),
                sim_require_finite=True,
                sim_require_nnan=True,
                nc=nc,
            )
        )

    devices = jax.devices()[:NCORES]
    mesh = Mesh(np.asarray(devices), ("core",))
    nin = len(in_names) + len(out_names)
    sharded = jax.jit(
        shard_map(
            _body,
            mesh=mesh,
            in_specs=(PartitionSpec("core"),) * nin,
            out_specs=(PartitionSpec("core"),) * len(out_names),
            check_rep=False,
        ),
        keep_unused=True,
    )
    concat_zeros = [
        np.zeros((NCORES * z.shape[0], *z.shape[1:]), z.dtype) for z in zero_outs
    ]
    _cache["runner"] = (sharded, in_names, out_names, concat_zeros)
    return _cache["runner"]


def kernel(x, token_positions, WQ, WK, WV, WO):
    in_maps = _make_in_maps(
        {
            "x": x,
            "token_positions": token_positions,
            "WQ": WQ,
            "WK": WK,
            "WV": WV,
            "WO": WO,
        }
    )
    sharded, in_names, out_names, concat_zeros = _get_runner()
    concat_in = [
        np.concatenate([np.asarray(in_maps[c][nm]) for c in range(NCORES)], axis=0)
        for nm in in_names
    ]
    out_arrs = sharded(*concat_in, *concat_zeros)
    ys = np.asarray(out_arrs[out_names.index("y")]).reshape(NCORES, T, D)
    out = np.empty((B, T, D), np.float32)
    for b in range(B):
        out[b] = ys[2 * b] + ys[2 * b + 1]
    return out


# revision 5
# speedup vs baseline: 2.5013x; 1.0530x over previous
"""Causal multi-head self-attention with RoPE on 8 NeuronCores.

Sharding (hardcoded): core c -> batch b = c // 2, head-group hg = c % 2.
Each core:
  - projects its batch's x with column-sharded WQ/WK/WV (8 heads = 512 dims),
  - applies RoPE (host-precomputed cos/sin tables shipped as bf16 hi/lo
    pairs, summed on-chip to f32; adjacent-pair swap via stream_shuffle),
  - runs causal attention for its 8 heads in transposed layout
    (S^T = [k, q]; softmax denominator comes free from a ones-column
    appended to V; normalization is broadcast via a DRAM bounce),
  - applies the row-sharded WO projection -> partial [T, D] output.
Host sums the two partials per batch (the "all-reduce after WO").

All inputs are packed into ONE DRAM tensor per core: the per-call
execution overhead through the axon PJRT path scales with the number of
I/O buffers (~65us each), so 8 separate inputs cost ~0.5ms of pure
launch overhead. Packed layout (bf16, [3200, 2048]):
  rows    0:1024  xT (x[b].T, [d, t])
  rows 1024:2048  wqT | wkT | wvT  (cols 0:512, 512:1024, 1024:1536)
  rows 2048:2560  woT (cols 0:1024)
  rows 2560:3072  cos_hi | cos_lo | sin_hi | sin_lo ([128, 2048] each)
  rows 3072:3200  trim mask (cols 0:128)
"""

import numpy as np
import ml_dtypes

B, T, D, H = 4, 2048, 1024, 16
DK = 64
HLOC = 8          # heads per core
E = HLOC * DK     # 512, local projection width
NCORES = 8
THETA = 10000.0

_BF16 = ml_dtypes.bfloat16

# packed-input row offsets
_ROW_X = 0
_ROW_W = 1024
_ROW_WO = 2048
_ROW_TBL = 2560
_ROW_TRIM = 3072
_PK_ROWS = 3200
_PK_COLS = 2048

_cache = {}


def _build(t=T, hloc=HLOC, d=D, reps=1, psum_layout=0):
    from contextlib import ExitStack

    import concourse.bacc as bacc
    import concourse.bass as bass  # noqa: F401
    import concourse.mybir as mybir
    import concourse.tile as tile

    f32 = mybir.dt.float32
    bf16 = mybir.dt.bfloat16
    Exp = mybir.ActivationFunctionType.Exp

    e = hloc * DK
    npair = hloc // 2       # head-pair tiles in QT/KT/OT
    dsub = d // 128         # contraction subtiles for projections
    tq = t // 512           # 512-wide q chunks
    tk = t // 128           # 128-wide k tiles
    swap_mask = [i ^ 1 for i in range(32)]
    e_v = min(512, e)

    nc = bacc.Bacc(
        "TRN2", target_bir_lowering=False, debug=False, enable_partition_id=False
    )

    pk = nc.declare_dram_parameter("pk", [_PK_ROWS, _PK_COLS], bf16, False).ap()
    y = nc.declare_dram_parameter("y", [t, d], f32, True).ap()
    scr = nc.dram_tensor("scr", [hloc, t], bf16)  # denom-recip bounce for bcast

    with tile.TileContext(nc) as tc:
        with ExitStack() as ctx:
            const = ctx.enter_context(tc.tile_pool(name="const", bufs=1))
            ptpool = ctx.enter_context(tc.tile_pool(name="ptp", bufs=8))
            normp = ctx.enter_context(tc.tile_pool(name="normp", bufs=4))
            ysbp = ctx.enter_context(tc.tile_pool(name="ysbp", bufs=3))

            wq_sb = const.tile([128, dsub, e], bf16)
            wk_sb = const.tile([128, dsub, e], bf16)
            wv_sb = const.tile([128, dsub, e], bf16)
            wo_sb = const.tile([128, e // 128, d], bf16)
            trim_sb = const.tile([128, 128], bf16)
            qt_sb = const.tile([128, npair, t], bf16)
            kt_sb = const.tile([128, npair, t], bf16)
            v_sb = const.tile([128, tk, hloc, DK + 1], bf16)
            ot_sb = const.tile([128, npair, t], bf16)

            # split the big constant loads per d-subtile so they spread
            # across DMA queues (a single dma_start rides one queue) and so
            # the first projection matmuls can start after ~0.5 MB
            xt_sb = const.tile([128, dsub, t], bf16)
            cos_sb = const.tile([128, t], f32)
            sin_sb = const.tile([128, t], f32)
            for ds_ in range(dsub):
                r = _ROW_W + ds_ * 128
                nc.sync.dma_start(wq_sb[:, ds_, :], pk[r : r + 128, 0:e])
                nc.sync.dma_start(
                    xt_sb[:, ds_, :], pk[ds_ * 128 : (ds_ + 1) * 128, :]
                )
                nc.sync.dma_start(wk_sb[:, ds_, :], pk[r : r + 128, e : 2 * e])
                nc.sync.dma_start(wv_sb[:, ds_, :], pk[r : r + 128, 2 * e : 3 * e])
            for dp in range(e // 128):
                r = _ROW_WO + dp * 128
                nc.sync.dma_start(wo_sb[:, dp, :], pk[r : r + 128, 0:d])
            nc.sync.dma_start(trim_sb, pk[_ROW_TRIM : _ROW_TRIM + 128, 0:128])
            nc.vector.memset(v_sb[:, :, :, DK : DK + 1], 1.0)
            with tc.tile_pool(name="tblp", bufs=1) as tblp:
                for dst, blk in ((cos_sb, 0), (sin_sb, 2)):
                    hi_sb = tblp.tile([128, t], bf16, tag="hi")
                    lo_sb = tblp.tile([128, t], bf16, tag="lo")
                    r = _ROW_TBL + blk * 128
                    nc.sync.dma_start(hi_sb, pk[r : r + 128, :])
                    nc.sync.dma_start(lo_sb, pk[r + 128 : r + 256, :])
                    nc.vector.tensor_add(dst, hi_sb, lo_sb)

            for _rep in range(reps):
                with ExitStack() as c1:
                    rope = c1.enter_context(tc.tile_pool(name="rope", bufs=3))
                    if psum_layout == 0:
                        ppsum = c1.enter_context(
                            tc.tile_pool(name="ppsum", bufs=1, space="PSUM")
                        )
                        spsum = c1.enter_context(
                            tc.tile_pool(name="spsum", bufs=2, space="PSUM")
                        )
                        opsum = c1.enter_context(
                            tc.tile_pool(name="opsum", bufs=2, space="PSUM")
                        )
                    elif psum_layout == 1:
                        spsum = c1.enter_context(
                            tc.tile_pool(name="spsum", bufs=2, space="PSUM")
                        )
                        opsum = c1.enter_context(
                            tc.tile_pool(name="opsum", bufs=4, space="PSUM")
                        )
                        ppsum = spsum
                    else:
                        spsum = c1.enter_context(
                            tc.tile_pool(name="spsum", bufs=3, space="PSUM")
                        )
                        opsum = c1.enter_context(
                            tc.tile_pool(name="opsum", bufs=2, space="PSUM")
                        )
                        ppsum = spsum

                    # ---- V projection (psum via the shared "s" slots) ----
                    for it in range(tk):
                        v_ps = spsum.tile([128, 1024], f32, tag="s")
                        for ds_ in range(dsub):
                            nc.tensor.matmul(
                                v_ps[:, :e_v],
                                lhsT=xt_sb[:, ds_, it * 128 : (it + 1) * 128],
                                rhs=wv_sb[:, ds_, :e_v],
                                start=(ds_ == 0),
                                stop=(ds_ == dsub - 1),
                            )
                        nc.vector.tensor_copy(
                            v_sb[:, it, :, 0:DK],
                            v_ps[:, :e_v].rearrange("p (h k) -> p h k", h=hloc),
                        )

                    ph = min(1024, t)          # projection t-chunk
                    nh = t // ph               # chunks per e-tile
                    for pr in range(npair):
                        # ---- Q/K projection + RoPE for this head pair ----
                        for wsb, dst in ((wq_sb, qt_sb), (wk_sb, kt_sb)):
                            for ch in range(nh):
                                q_ps = ppsum.tile(
                                    [128, ph],
                                    f32,
                                    tag="proj" if psum_layout == 0 else "s",
                                )
                                c0 = ch * ph
                                for ds_ in range(dsub):
                                    for jt in range(ph // 512):
                                        lo = c0 + jt * 512
                                        nc.tensor.matmul(
                                            q_ps[:, jt * 512 : (jt + 1) * 512],
                                            lhsT=wsb[:, ds_, pr * 128 : (pr + 1) * 128],
                                            rhs=xt_sb[:, ds_, lo : lo + 512],
                                            start=(ds_ == 0),
                                            stop=(ds_ == dsub - 1),
                                        )
                                # evict raw proj to sbuf, then RoPE from sbuf
                                raw = rope.tile([128, ph], f32, tag="raw")
                                nc.vector.tensor_copy(raw, q_ps[:, :])
                                sw = rope.tile([128, ph], f32, tag="sw")
                                nc.vector.stream_shuffle(sw, raw, mask=swap_mask)
                                nc.vector.tensor_mul(
                                    dst[:, pr, c0 : c0 + ph],
                                    raw,
                                    cos_sb[:, c0 : c0 + ph],
                                )
                                nc.gpsimd.tensor_mul(sw, sw, sin_sb[:, c0 : c0 + ph])
                                nc.gpsimd.tensor_add(
                                    dst[:, pr, c0 : c0 + ph],
                                    dst[:, pr, c0 : c0 + ph],
                                    sw,
                                )

                        # ---- attention for heads (2pr, 2pr+1), q-subtiles ----
                        hA, hB = 2 * pr, 2 * pr + 1
                        for j in range(tq):
                            o_a = opsum.tile([128, 512], f32, tag="o")
                            o_b = opsum.tile([128, 512], f32, tag="o")
                            for i in range(4 * j + 4):
                                off = 128 * (i % 4) if i // 4 == j else 0
                                qlo = j * 512
                                s_ps = spsum.tile([128, 1024], f32, tag="s")
                                # diagonal tiles: stream only q-columns >= off
                                # (below-diagonal columns are all-zero in P
                                # and never touched anywhere downstream)
                                nc.tensor.matmul(
                                    s_ps[:, off:512],
                                    lhsT=kt_sb[0:64, pr, i * 128 : (i + 1) * 128],
                                    rhs=qt_sb[0:64, pr, qlo + off : (j + 1) * 512],
                                    start=True,
                                    stop=True,
                                )
                                nc.tensor.matmul(
                                    s_ps[:, 512 + off : 1024],
                                    lhsT=kt_sb[64:128, pr, i * 128 : (i + 1) * 128],
                                    rhs=qt_sb[64:128, pr, qlo + off : (j + 1) * 512],
                                    start=True,
                                    stop=True,
                                )
                                pt = ptpool.tile([128, 1024], bf16, tag="pt")
                                if off == 0:
                                    nc.scalar.activation(
                                        pt[:, :], s_ps[:, :], Exp, scale=DK ** (-0.5)
                                    )
                                else:
                                    nc.scalar.activation(
                                        pt[:, off:512],
                                        s_ps[:, off:512],
                                        Exp,
                                        scale=DK ** (-0.5),
                                    )
                                    nc.scalar.activation(
                                        pt[:, 512 + off : 1024],
                                        s_ps[:, 512 + off : 1024],
                                        Exp,
                                        scale=DK ** (-0.5),
                                    )
                                if i // 4 == j:
                                    nc.gpsimd.tensor_mul(
                                        pt[:, off : off + 128],
                                        pt[:, off : off + 128],
                                        trim_sb,
                                    )
                                    nc.gpsimd.tensor_mul(
                                        pt[:, 512 + off : 512 + off + 128],
                                        pt[:, 512 + off : 512 + off + 128],
                                        trim_sb,
                                    )
                                nc.tensor.matmul(
                                    o_a[0:65, off:512],
                                    lhsT=v_sb[:, i, hA, :],
                                    rhs=pt[:, off:512],
                                    start=(i == 0),
                                    stop=(i == 4 * j + 3),
                                )
                                nc.tensor.matmul(
                                    o_b[0:65, off:512],
                                    lhsT=v_sb[:, i, hB, :],
                                    rhs=pt[:, 512 + off : 1024],
                                    start=(i == 0),
                                    stop=(i == 4 * j + 3),
                                )
                            # normalize both heads for this q-subtile
                            for hx, o_ps, po in ((hA, o_a, 0), (hB, o_b, 64)):
                                recip = normp.tile([1, 512], bf16, tag="recip")
                                with nc.allow_low_precision(
                                    reason="softmax recip bf16"
                                ):
                                    nc.vector.reciprocal(
                                        recip[0:1, :], o_ps[64:65, 0:512]
                                    )
                                nc.sync.dma_start(
                                    scr[hx : hx + 1, j * 512 : (j + 1) * 512],
                                    recip[0:1, :],
                                )
                                bc_sb = normp.tile([64, 512], bf16, tag="bc")
                                scr_bcast = bass.AP(
                                    tensor=scr.ap().tensor,
                                    offset=hx * t + j * 512,
                                    ap=[[0, 64], [1, 512]],
                                )
                                nc.sync.dma_start(bc_sb, scr_bcast)
                                nc.vector.tensor_mul(
                                    ot_sb[po : po + 64, pr, j * 512 : (j + 1) * 512],
                                    o_ps[0:64, 0:512],
                                    bc_sb,
                                )

                    # ---- WO projection (psum via the shared "s" slots) ----
                    nech = d // 512
                    for it in range(tk):
                        y_sb = ysbp.tile([128, d], f32, tag="ysb")
                        for ec in range(nech):
                            y_ps = spsum.tile([128, 1024], f32, tag="s")
                            for dp in range(e // 128):
                                nc.tensor.matmul(
                                    y_ps[:, 0:512],
                                    lhsT=ot_sb[:, dp, it * 128 : (it + 1) * 128],
                                    rhs=wo_sb[:, dp, ec * 512 : (ec + 1) * 512],
                                    start=(dp == 0),
                                    stop=(dp == e // 128 - 1),
                                )
                            nc.vector.tensor_copy(
                                y_sb[:, ec * 512 : (ec + 1) * 512], y_ps[:, 0:512]
                            )
                        nc.sync.dma_start(y[it * 128 : (it + 1) * 128, :], y_sb)

    nc.compile()
    return nc


def _get_nc():
    if "nc" not in _cache:
        _cache["nc"] = _build()
    return _cache["nc"]


def _host_tables(positions):
    """cos/sin RoPE tables laid out for the on-chip [128, T] tiles."""
    pos = np.asarray(positions, np.float32)  # [t]
    inv = 1.0 / THETA ** (
        (2.0 * np.arange(1, DK // 2 + 1, dtype=np.float32) - 2.0) / DK
    )  # [32]
    ang = pos[None, :] * inv[:, None]  # [32, t]
    c32 = np.cos(ang)
    s32 = np.sin(ang)
    rows = np.arange(128)
    dloc = rows % DK
    fidx = dloc // 2
    sign = np.where(dloc % 2 == 0, -1.0, 1.0).astype(np.float32)
    cosT = c32[fidx, :]
    sinT = sign[:, None] * s32[fidx, :]
    return np.ascontiguousarray(cosT), np.ascontiguousarray(sinT)


def _hi_lo(a):
    hi = a.astype(_BF16)
    lo = (a - hi.astype(np.float32)).astype(_BF16)
    return hi, lo


def _make_in_maps(inputs):
    x = np.asarray(inputs["x"], np.float32)
    token_positions = np.asarray(inputs["token_positions"])
    WQ = np.asarray(inputs["WQ"], np.float32)
    WK = np.asarray(inputs["WK"], np.float32)
    WV = np.asarray(inputs["WV"], np.float32)
    WO = np.asarray(inputs["WO"], np.float32)
    trimask = np.triu(np.ones((128, 128), np.float32)).astype(_BF16)

    # per-head-group weight shards (shared across batches)
    wsh = {}
    for hg in range(2):
        sl = slice(hg * E, (hg + 1) * E)
        wsh[hg] = (
            np.ascontiguousarray(WQ[sl, :].T).astype(_BF16),
            np.ascontiguousarray(WK[sl, :].T).astype(_BF16),
            np.ascontiguousarray(WV[sl, :].T).astype(_BF16),
            np.ascontiguousarray(WO[:, sl].T).astype(_BF16),
        )
    xts = {b: np.ascontiguousarray(x[b].T).astype(_BF16) for b in range(B)}
    tabs = {}
    for b in range(B):
        key = token_positions[b].tobytes()
        if key not in tabs:
            cosT, sinT = _host_tables(token_positions[b])
            tabs[key] = (*_hi_lo(cosT), *_hi_lo(sinT))

    in_maps = []
    for c in range(NCORES):
        b, hg = c // 2, c % 2
        chi, clo, shi, slo = tabs[token_positions[b].tobytes()]
        wq, wk, wv, wo = wsh[hg]
        pkb = np.zeros((_PK_ROWS, _PK_COLS), _BF16)
        pkb[_ROW_X : _ROW_X + D, :] = xts[b]
        pkb[_ROW_W : _ROW_W + D, 0:E] = wq
        pkb[_ROW_W : _ROW_W + D, E : 2 * E] = wk
        pkb[_ROW_W : _ROW_W + D, 2 * E : 3 * E] = wv
        pkb[_ROW_WO : _ROW_WO + E, 0:D] = wo
        pkb[_ROW_TBL : _ROW_TBL + 128, :] = chi
        pkb[_ROW_TBL + 128 : _ROW_TBL + 256, :] = clo
        pkb[_ROW_TBL + 256 : _ROW_TBL + 384, :] = shi
        pkb[_ROW_TBL + 384 : _ROW_TBL + 512, :] = slo
        pkb[_ROW_TRIM : _ROW_TRIM + 128, 0:128] = trimask
        in_maps.append({"pk": pkb})
    return in_maps


def _get_runner():
    """Build (once) a jitted shard_map over the 8 cores for the bass program."""
    if "runner" in _cache:
        return _cache["runner"]

    import jax
    from jax.sharding import Mesh, PartitionSpec
    from jax.experimental.shard_map import shard_map
    from concourse import bass2jax
    from concourse.bass2jax import _bass_exec_p, partition_id_tensor
    import concourse.mybir as mybir

    bass2jax.install_neuronx_cc_hook()
    nc = _get_nc()

    partition_name = nc.partition_id_tensor.name if nc.partition_id_tensor else None
    in_names, out_names, out_avals, zero_outs = [], [], [], []
    for alloc in nc.m.functions[0].allocations:
        if not isinstance(alloc, mybir.MemoryLocationSet):
            continue
        name = alloc.memorylocations[0].name
        if alloc.kind == "ExternalInput":
            if name != partition_name:
                in_names.append(name)
        elif alloc.kind == "ExternalOutput":
            out_names.append(name)
            np_dt = mybir.dt.np(alloc.dtype)
            out_avals.append(jax.core.ShapedArray(tuple(alloc.tensor_shape), np_dt))
            zero_outs.append(np.zeros(tuple(alloc.tensor_shape), np_dt))
    all_names = in_names + out_names
    if partition_name is not None:
        all_names = all_names + [partition_name]

    def _body(*args):
        operands = list(args)
        if partition_name is not None:
            operands.append(partition_id_tensor())
        return tuple(
            _bass_exec_p.bind(
                *operands,
                out_avals=tuple(out_avals),
                in_names=tuple(all_names),
                out_names=tuple(out_names),
                lowering_input_output_aliases=(),
                sim_require_finite=True,
                sim_require_nnan=True,
                nc=nc,
            )
        )

    devices = jax.devices()[:NCORES]
    mesh = Mesh(np.asarray(devices), ("core",))
    nin = len(in_names) + len(out_names)
    sharded = jax.jit(
        shard_map(
            _body,
            mesh=mesh,
            in_specs=(PartitionSpec("core"),) * nin,
            out_specs=(PartitionSpec("core"),) * len(out_names),
            check_rep=False,
        ),
        keep_unused=True,
    )
    concat_zeros = [
        np.zeros((NCORES * z.shape[0], *z.shape[1:]), z.dtype) for z in zero_outs
    ]
    _cache["runner"] = (sharded, in_names, out_names, concat_zeros)
    return _cache["runner"]


def kernel(x, token_positions, WQ, WK, WV, WO):
    in_maps = _make_in_maps(
        {
            "x": x,
            "token_positions": token_positions,
            "WQ": WQ,
            "WK": WK,
            "WV": WV,
            "WO": WO,
        }
    )
    sharded, in_names, out_names, concat_zeros = _get_runner()
    concat_in = [
        np.concatenate([np.asarray(in_maps[c][nm]) for c in range(NCORES)], axis=0)
        for nm in in_names
    ]
    out_arrs = sharded(*concat_in, *concat_zeros)
    ys = np.asarray(out_arrs[out_names.index("y")]).reshape(NCORES, T, D)
    out = np.empty((B, T, D), np.float32)
    for b in range(B):
        out[b] = ys[2 * b] + ys[2 * b + 1]
    return out
